# revision 1
# baseline (speedup 1.0000x reference)
"""Haar DWT (2x2 stride-2 block decomposition) on 8 Trainium2 NeuronCores.

Input x: (32, 3, 512, 512) f32. Outputs (ll, lh, hl, hh): each (32, 3, 256, 256).

Sharding: pure data parallel over the batch dim — 4 images per core.
Per core the shard is viewed as 12 channel images of 512x512. Each channel
is one SBUF tile: 128 partitions x (4 rows of 512) — an 8 KB/partition fully
contiguous 1 MB load DMA. Vertical pair sum/diff run on DVE via strided views,
the x0.5 scale runs in-place on ACT, and the horizontal (stride-2 column)
combines produce contiguous 2 KB/partition output tiles stored with fully
contiguous 256 KB DMAs.
"""

import sys

import numpy as np

if "/opt/trn_rl_repo" not in sys.path:
    sys.path.insert(0, "/opt/trn_rl_repo")

from concourse import bacc, mybir
from concourse import tile
from concourse.bass_utils import run_bass_kernel_spmd

N_CORES = 8
B, C, H, W = 32, 3, 512, 512
BPC = B // N_CORES  # images per core
NCH = BPC * C  # channel images per core
P = 128  # SBUF partitions
RPP = H // P  # rows per partition (4)
HW_OUT = H // 2  # 256

_CACHE = {}


def _build():
    nc = bacc.Bacc("TRN2", target_bir_lowering=False, debug=False)
    f32 = mybir.dt.float32
    x = nc.dram_tensor("x", [NCH, P, RPP, W], f32, kind="ExternalInput")
    onames = ("ll", "lh", "hl", "hh")
    outs = {
        nm: nc.dram_tensor(nm, [NCH, P, 2, HW_OUT], f32, kind="ExternalOutput")
        for nm in onames
    }
    with tile.TileContext(nc) as tc:
        with tc.tile_pool(name="p", bufs=3) as pool:
            for i in range(NCH):
                xin = pool.tile([P, 2, 2, W], f32)
                nc.sync.dma_start(
                    out=xin[:].rearrange("p a b c -> p (a b) c"), in_=x.ap()[i]
                )
                e = xin[:, :, 0, :]  # even image rows
                o = xin[:, :, 1, :]  # odd image rows
                su = pool.tile([P, 2, W], f32)
                df = pool.tile([P, 2, W], f32)
                nc.vector.tensor_add(su[:], e, o)  # a+c / b+d interleaved
                nc.vector.tensor_sub(df[:], o, e)  # c-a / d-b interleaved
                nc.scalar.mul(su[:], su[:], 0.5)
                nc.scalar.mul(df[:], df[:], 0.5)
                sv = su[:].rearrange("p k (j t) -> p k j t", t=2)
                dv = df[:].rearrange("p k (j t) -> p k j t", t=2)
                ll = pool.tile([P, 2, HW_OUT], f32)
                lh = pool.tile([P, 2, HW_OUT], f32)
                hl = pool.tile([P, 2, HW_OUT], f32)
                hh = pool.tile([P, 2, HW_OUT], f32)
                nc.vector.tensor_add(ll[:], sv[:, :, :, 0], sv[:, :, :, 1])
                nc.vector.tensor_sub(hl[:], sv[:, :, :, 1], sv[:, :, :, 0])
                nc.vector.tensor_add(lh[:], dv[:, :, :, 0], dv[:, :, :, 1])
                nc.vector.tensor_sub(hh[:], dv[:, :, :, 1], dv[:, :, :, 0])
                for nm, t in (("ll", ll), ("lh", lh), ("hl", hl), ("hh", hh)):
                    nc.sync.dma_start(out=outs[nm].ap()[i], in_=t[:])
    nc.compile()
    return nc


def _get_nc():
    if "nc" not in _CACHE:
        _CACHE["nc"] = _build()
    return _CACHE["nc"]


def run(x, **spmd_kwargs):
    """Run the DWT on 8 cores; returns (results_tuple, BassKernelResults)."""
    nc = _get_nc()
    xs = np.ascontiguousarray(np.asarray(x, dtype=np.float32)).reshape(
        N_CORES, NCH, P, RPP, W
    )
    in_maps = [{"x": xs[i]} for i in range(N_CORES)]
    res = run_bass_kernel_spmd(nc, in_maps, core_ids=list(range(N_CORES)), **spmd_kwargs)
    out = []
    for nm in ("ll", "lh", "hl", "hh"):
        arr = np.stack([res.results[i][nm] for i in range(N_CORES)])
        out.append(arr.reshape(B, C, HW_OUT, HW_OUT))
    return tuple(out), res


def kernel(x):
    out, _ = run(x)
    return out
